# revision 26
# baseline (speedup 1.0000x reference)
"""TopK sparse autoencoder forward pass on 8 Trainium2 NeuronCores.

Math (reference):
    preact = (x - b_dec) @ W_enc.T + b_enc          # [B, F]
    top32 = exact per-row top-32 of relu(preact)
    x_hat = scatter(top32) @ W_dec.T + b_dec        # [B, D]

Strategy: data-parallel over batch rows (1024 rows/core, no collectives).
Per core:
  encode: f32r matmul (PE, full rate at free-dim>=256); f-block-outer
          loop; PSUM -> stage (Act) -> DRAM fp32 spill + DVE chunk-max
          (width 32) into cm tiles.
  T1a (select): top-32 chunks via DVE max/max_index/match_replace rounds
          on cm [128,512]; issue 32 single-offset indirect DMAs (walrus
          in this container mislowers multi-offset forms and cannot load
          gpsimd libraries, so batched dma_gather is unavailable; each
          indirect DMA costs ~1us fixed on the Pool engine).
  T1b: 4 more DVE rounds on the 1024 gathered candidates give exact
          top-32 vals + global idx.
  T2a: build diag(vals) tiles (DVE) and pre-issue the first wdec-row
          gathers; T2b: remaining gathers + sum_c diag(vals[:,c]) @ G_c
          accumulated on the PE, then bias/copy + store.
Pipelining: engines are in-order, so the stages are software-pipelined
across b-tiles (gather of tile i+1 in flight while tile i runs its
DVE exact phase; PE stream stays pure encode until the group's encode
is done). Groups restream W_enc; earlier groups' tails hide inside the
next group's encode window.
"""
import sys
sys.path.insert(0, '/opt/trn_rl_repo')

import numpy as np

B, D, F, K = 8192, 768, 16384, 32
N_CORES = 8
BC = B // N_CORES          # rows per core (1024)
NBT = BC // 128            # b-tiles per core (8)
NFB = F // 512             # f-blocks (32)
KD = D // 128              # contraction chunks (6)
CH = 32                    # topk chunk width
C = F // CH                # chunks per row (512)
SH = CH.bit_length() - 1   # log2(CH)
NCH = K // 8               # rounds of 8 (4)

ENC_MODE = "f32r"          # "fp32" | "f32r"
TAIL_MODE = "dfirst"
GROUPS = (8,)              # b-tile group sizes (sum = NBT)
GP_BUFS = 10               # wdec gather buffers
PRE_G = 7                  # gathers pre-issued in t2a (2*PRE_G <= GP_BUFS)

_cache = {}


def _fix_sync_waits(nc, maxw=1):
    """This container's walrus rejects >1 sync wait per instruction; split
    excess waits onto same-engine NoOps inserted just before."""
    import bass_rust
    import concourse.mybir as mybir
    ctr = 0
    for f in nc.m.functions:
        for bb in f.blocks:
            out, changed = [], False
            for inst in bb.instructions:
                si = inst.sync_info
                waits = list(si.on_wait) if si is not None else []
                if len(waits) > maxw:
                    changed = True
                    head, keep = waits[:-maxw], waits[-maxw:]
                    for i in range(0, len(head), maxw):
                        ctr += 1
                        nop = mybir.InstNoOp(
                            name=f"syncfix-nop-{id(nc)}-{ctr}", ins=[], outs=[])
                        nop.engine = inst.engine
                        nop.sync_info = bass_rust.SyncInfo(
                            on_wait=head[i:i + maxw], on_update=[])
                        out.append(nop)
                    si.on_wait = keep
                out.append(inst)
            if changed:
                bb.instructions = out


def _build(has_benc: bool, has_bdec: bool, repeat: int = 1):
    import concourse.bass as bass
    import concourse.mybir as mybir
    import concourse.tile as tile
    dt = mybir.dt
    Alu = mybir.AluOpType

    nc = bass.Bass("TRN2", target_bir_lowering=False, debug=False,
                   num_devices=N_CORES)

    _edt = dt.float32r if ENC_MODE == "f32r" else dt.float32
    xT_d = nc.dram_tensor("xT", [D, BC], _edt, kind="ExternalInput")
    wencT_d = nc.dram_tensor("wencT", [D, F], _edt, kind="ExternalInput")
    wdecT_d = nc.dram_tensor("wdecT16", [F, D], dt.float16, kind="ExternalInput")
    beff_d = nc.dram_tensor("beff", [1, F], dt.float32, kind="ExternalInput")
    bdec_d = nc.dram_tensor("bdec", [1, D], dt.float32, kind="ExternalInput")
    out_d = nc.dram_tensor("xhat", [BC, D], dt.float32, kind="ExternalOutput")
    # one spill tensor per b-tile: gathers never falsely conflict (WAR)
    # with another tile's spill writes in the dep tracker
    preact_ds = [nc.dram_tensor(f"preact_spill{bt}", [128, F], dt.float32)
                 for bt in range(NBT)]
    preact_flats = [p.ap().rearrange("p (c w) -> (p c) w", w=CH)
                    for p in preact_ds]

    def body(tc, pools):
        sb, sb1, stp, psA, psB, gp, wbp = pools

        # resident inputs
        xT = sb1.tile([128, KD, BC],
                      dt.float32r if ENC_MODE == "f32r" else dt.float32)
        nc.sync.dma_start(
            xT[:], xT_d.ap().rearrange("(po pi) b -> pi po b", pi=128))
        if has_benc:
            beff = sb1.tile([1, F], dt.float32)
            nc.sync.dma_start(beff[:], beff_d.ap())
        if has_bdec:
            bdec1 = sb1.tile([1, D], dt.float32)
            nc.sync.dma_start(bdec1[:], bdec_d.ap())
            bdec_bc = sb1.tile([128, D], dt.float32)
            nc.gpsimd.partition_broadcast(bdec_bc[:], bdec1[:])

        cms = [sb1.tile([128, C], dt.float32, name=f"cm{bt}")
               for bt in range(NBT)]
        maxg = max(GROUPS)
        stgq = [[sb1.tile([128, 2, 512], dt.float32, name=f"stgq{loc}_{j}")
                 for j in range(2)] for loc in range(maxg)]
        loc_of = {}
        s0 = 0
        for n in GROUPS:
            for j in range(n):
                loc_of[s0 + j] = j
            s0 += n
        valss = [sb1.tile([128, K], dt.float32, name=f"vals{bt}")
                 for bt in range(NBT)]
        idxss = [sb1.tile([128, K], dt.uint32, name=f"idx{bt}")
                 for bt in range(NBT)]

        # iota constants (standard gpsimd library, preloaded)
        jiota = sb1.tile([128, K], dt.uint32)
        nc.gpsimd.iota(jiota[:], pattern=[[1, K]], base=0, channel_multiplier=0)
        # fp16 identity mask for building diag(vals) tiles on DVE
        iota_p = sb1.tile([128, 1], dt.uint32)
        nc.gpsimd.iota(iota_p[:], pattern=[[1, 1]], base=0, channel_multiplier=1)
        iota_f = sb1.tile([128, 128], dt.uint32)
        nc.gpsimd.iota(iota_f[:], pattern=[[1, 128]], base=0,
                       channel_multiplier=0)
        idmask = sb1.tile([128, 128], dt.float16)
        nc.vector.tensor_tensor(idmask[:],
                                iota_p[:, :1].to_broadcast([128, 128]),
                                iota_f[:], op=Alu.is_equal)
        # per-partition row offset into the per-tile gather table (p*C)
        rowoff = sb1.tile([128, 1], dt.uint32)
        nc.gpsimd.iota(rowoff[:], pattern=[[1, 1]], base=0,
                       channel_multiplier=C)

        # ---------------- encode + spill + chunk-max ----------------
        wencT_v = wencT_d.ap().rearrange("(po pi) f -> pi po f", pi=128)

        def encode_fb(fb, bts):
            wb = wbp.tile([128, KD, 512],
                          dt.float32r if ENC_MODE == "f32r" else dt.float32,
                          tag="wb")
            nc.sync.dma_start(wb[:], wencT_v[:, :, fb * 512:(fb + 1) * 512])
            if has_benc:
                beff_bc = stp.tile([128, 512], dt.float32, tag="beffbc")
                nc.gpsimd.partition_broadcast(
                    beff_bc[:], beff[:, fb * 512:(fb + 1) * 512])
            for bt in bts:
                ps = psA.tile([128, 512], dt.float32, tag="encps")
                for k in range(KD):
                    lhsT = xT[:, k, bt * 128:(bt + 1) * 128]
                    rhs = wb[:, k, :]
                    nc.tensor.matmul(ps[:], lhsT=lhsT, rhs=rhs,
                                     start=(k == 0), stop=(k == KD - 1))
                # evacuate PSUM via Act only (frees the bank fast; DVE
                # chunk-max reads the SBUF stage copy so encode never
                # stalls on DVE backlog). Two f-blocks accumulate in a
                # per-slot quad buffer; one 8KB-line spill DMA per quad.
                q = fb % 2
                sq = stgq[loc_of[bt]][(fb // 2) % 2]
                stage = sq[:, q]
                if has_benc:
                    nc.vector.tensor_add(stage, ps[:], beff_bc[:])
                else:
                    nc.scalar.copy(stage, ps[:])
                nc.vector.tensor_reduce(
                    cms[bt][:, fb * (512 // CH):(fb + 1) * (512 // CH)],
                    stage.rearrange("p (c w) -> p c w", w=CH),
                    axis=mybir.AxisListType.X, op=Alu.max)
                if q == 1:
                    nc.sync.dma_start(
                        preact_ds[bt].ap()[:, (fb - 1) * 512:(fb + 1) * 512],
                        sq[:].rearrange("p a b -> p (a b)"))

        # ---------------- T1: per-b-tile topk selection ----------------
        t1_state = {}

        def tail_select_a(bt):
            # top-32 chunks by chunk max; issue the candidate gathers
            cmw = sb.tile([128, C], dt.float32, tag="cmw")
            nc.vector.tensor_copy(cmw[:], cms[bt][:])
            cm8 = sb.tile([128, 8], dt.float32, tag="cm8")
            chunkid = sb.tile([128, K], dt.uint32, tag="chunkid",
                              name=f"chunkid{bt}")
            for r in range(NCH):
                nc.vector.max(out=cm8[:], in_=cmw[:])
                nc.vector.max_index(out=chunkid[:, r * 8:(r + 1) * 8],
                                    in_max=cm8[:], in_values=cmw[:])
                if r != NCH - 1:
                    nc.vector.match_replace(out=cmw[:], in_to_replace=cm8[:],
                                            in_values=cmw[:], imm_value=-1e30)

            off = sb.tile([128, K], dt.uint32, tag="off")
            nc.vector.tensor_tensor(off[:], chunkid[:],
                                    rowoff[:, :1].to_broadcast([128, K]),
                                    op=Alu.add)
            chunk16 = sb.tile([128, K], dt.uint16, tag="chunk16",
                              name=f"chunk16_{bt}")
            nc.scalar.copy(chunk16[:], chunkid[:])
            cand = sb.tile([128, K, CH], dt.float32, tag="cand",
                           name=f"cand{bt}")
            for j in range(K):
                nc.gpsimd.indirect_dma_start(
                    out=cand[:, j], out_offset=None,
                    in_=preact_flats[bt],
                    in_offset=bass.IndirectOffsetOnAxis(ap=off[:, j:j + 1],
                                                        axis=0))
            t1_state[bt] = (chunkid, chunk16, cand)

        def tail_select_b(bt):
            # exact top-32 of the gathered candidates
            chunkid, chunk16, cand = t1_state.pop(bt)
            candf = cand[:].rearrange("p a b -> p (a b)")
            vals = valss[bt]
            pos = sb.tile([128, K], dt.uint32, tag="pos")
            for r in range(NCH):
                m8 = vals[:, r * 8:(r + 1) * 8]
                nc.vector.max(out=m8, in_=candf)
                nc.vector.max_index(out=pos[:, r * 8:(r + 1) * 8],
                                    in_max=m8, in_values=candf)
                if r != NCH - 1:
                    nc.vector.match_replace(out=candf, in_to_replace=m8,
                                            in_values=candf, imm_value=-1e30)

            # positions -> global feature indices:
            # idx = chunkid[p, pos>>SH]*CH + (pos&(CH-1)); chunkid lookup via
            # one-hot compare-multiply-reduce (no per-partition gather on HW)
            j32 = sb.tile([128, K], dt.uint32, tag="j32")
            nc.vector.tensor_scalar(j32[:], pos[:], SH, None,
                                    op0=Alu.logical_shift_right)
            l32 = sb.tile([128, K], dt.uint32, tag="l32")
            nc.vector.tensor_scalar(l32[:], pos[:], CH - 1, None,
                                    op0=Alu.bitwise_and)
            eq = sb.tile([128, K, K], dt.uint16, tag="eq")
            nc.vector.tensor_tensor(
                eq[:], j32[:, :, None].to_broadcast([128, K, K]),
                jiota[:, None, :].to_broadcast([128, K, K]), op=Alu.is_equal)
            nc.vector.tensor_tensor(
                eq[:], eq[:], chunk16[:, None, :].to_broadcast([128, K, K]),
                op=Alu.mult)
            cs32 = sb.tile([128, K], dt.uint32, tag="cs32")
            nc.vector.tensor_reduce(cs32[:], eq[:],
                                    axis=mybir.AxisListType.X, op=Alu.max)
            idx32 = idxss[bt]
            nc.vector.tensor_scalar(idx32[:], cs32[:], SH, None,
                                    op0=Alu.logical_shift_left)
            nc.vector.tensor_tensor(idx32[:], idx32[:], l32[:], op=Alu.add)

            # relu guard (rows with <32 positive preacts: extra top-k entries
            # are relu zeros in the reference; zero coefficients match it)
            nc.vector.tensor_scalar_max(vals[:], vals[:], 0.0)

        # ---------------- T2: per-b-tile compact decode ----------------
        def tail_decode_b(bt):
            vals, idx32 = valss[bt], idxss[bt]
            # diag[p, c, :] = vals[p, c] * (identity row p)
            diag = sb.tile([128, K, 128], dt.float16, tag="diag",
                           name=f"diag{bt}")
            nc.vector.tensor_tensor(
                diag[:], vals[:, :, None].to_broadcast([128, K, 128]),
                idmask[:, None, :].to_broadcast([128, K, 128]), op=Alu.mult)
            pso = psB.tile([128, D], dt.float32, tag="decps")
            for c in range(K):
                g = gp.tile([128, D], dt.float16, tag="g")
                nc.gpsimd.indirect_dma_start(
                    out=g[:], out_offset=None,
                    in_=wdecT_d.ap(),
                    in_offset=bass.IndirectOffsetOnAxis(
                        ap=idx32[:, c:c + 1], axis=0))
                nc.tensor.matmul(pso[:, :512], lhsT=diag[:, c, :],
                                 rhs=g[:, :512],
                                 start=(c == 0), stop=(c == K - 1))
                nc.tensor.matmul(pso[:, 512:D], lhsT=diag[:, c, :],
                                 rhs=g[:, 512:D],
                                 start=(c == 0), stop=(c == K - 1))
            osb = stp.tile([128, D], dt.float32, tag="osb")
            if has_bdec:
                nc.vector.tensor_add(osb[:], pso[:], bdec_bc[:])
            else:
                nc.scalar.copy(osb[:], pso[:])
            nc.sync.dma_start(out_d.ap()[bt * 128:(bt + 1) * 128, :], osb[:])

        # ---------------- schedule ----------------
        groups, s = [], 0
        for n in GROUPS:
            groups.append(list(range(s, s + n)))
            s += n
        assert s == NBT

        stages = {"a": tail_select_a, "b": tail_select_b,
                  "d": tail_decode_b}
        prev, dq = [], []
        for bts in groups:
            # encode this group, interleaving the previous group's T1 stages
            # and T2 gather pre-issues between f-blocks, software-pipelined:
            # the candidate gathers of tile i+1 are in flight while tile i
            # runs its DVE exact phase. PE stream stays pure encode (decode
            # matmuls would stall the in-order PE queue on their gathers).
            ev = []
            for i, bt in enumerate(prev):
                ev.append(("a", bt))
                if i >= 1:
                    ev.append(("b", prev[i - 1]))
            if prev:
                ev.append(("b", prev[-1]))
            acts = {}
            if ev:
                step = max(1, (NFB - 1) // len(ev))
                for i, e in enumerate(ev):
                    acts.setdefault(1 + i * step, []).append(e)
            for fb in range(NFB):
                encode_fb(fb, bts)
                for kind, bt in acts.get(fb, ()):
                    stages[kind](bt)
            for kind, bt in [e for f, es in sorted(acts.items())
                             for e in es if f >= NFB]:
                stages[kind](bt)
            # decode of the group whose selection was just interleaved is
            # deferred into the final merged pipeline
            dq.extend(prev)
            prev = bts
        # final tail: interleave the deferred decodes with the last group's
        # selection pipeline so the Pool queue (gathers) never starves
        n = len(prev)
        ev = []
        if TAIL_MODE == "dfirst":
            # pending decodes first (Pool work immediately available), then
            # the last group's pipelined selection + its decodes
            ev.extend(("d", bt) for bt in dq)
            dq = []
            for i, bt in enumerate(prev):
                ev.append(("a", bt))
                if i >= 1:
                    ev.append(("b", prev[i - 1]))
            ev.append(("b", prev[-1]))
            ev.extend(("d", bt) for bt in prev)
        elif TAIL_MODE == "merge":
            for i, bt in enumerate(prev):
                ev.append(("a", bt))
                if i >= 1:
                    if dq:
                        ev.append(("d", dq.pop(0)))
                    ev.append(("b", prev[i - 1]))
                    dq.append(prev[i - 1])
                    if dq and i >= 2:
                        ev.append(("d", dq.pop(0)))
            ev.append(("b", prev[-1]))
            dq.append(prev[-1])
            ev.extend(("d", bt) for bt in dq)
        else:  # "simple": decodes of earlier groups, then last-group pipeline
            ev.extend(("d", bt) for bt in dq)
            dq = []
            for i in range(n + 2):
                if i < n:
                    ev.append(("a", prev[i]))
                if 1 <= i <= n:
                    ev.append(("b", prev[i - 1]))
                if i >= 2:
                    ev.append(("d", prev[i - 2]))
        for kind, bt in ev:
            stages[kind](bt)

    from contextlib import ExitStack
    with tile.TileContext(nc) as tc:
        with ExitStack() as ctx:
            pools = (
                ctx.enter_context(tc.tile_pool(name="sb", bufs=2)),
                ctx.enter_context(tc.tile_pool(name="sb1", bufs=1)),
                ctx.enter_context(tc.tile_pool(name="stage", bufs=3)),
                ctx.enter_context(tc.tile_pool(name="psA", bufs=6, space="PSUM")),
                ctx.enter_context(tc.tile_pool(name="psB", bufs=1, space="PSUM")),
                ctx.enter_context(tc.tile_pool(name="gpool", bufs=GP_BUFS)),
                ctx.enter_context(tc.tile_pool(name="wbpool", bufs=3)),
            )
            if repeat == 1:
                body(tc, pools)
            else:
                with tc.For_i(0, repeat, 1):
                    body(tc, pools)

    _fix_sync_waits(nc)
    return nc


def _get_runner(has_benc, has_bdec, repeat=1):
    key = (has_benc, has_bdec, repeat, ENC_MODE, GROUPS, CH, GP_BUFS, PRE_G)
    if key in _cache:
        return _cache[key]
    import jax
    from jax.sharding import Mesh, PartitionSpec
    from jax.experimental.shard_map import shard_map
    import concourse.mybir as mybir
    from concourse import bass2jax
    from concourse.bass2jax import _bass_exec_p, install_neuronx_cc_hook

    nc = _build(has_benc, has_bdec, repeat)
    install_neuronx_cc_hook()

    partition_name = (nc.partition_id_tensor.name
                      if nc.partition_id_tensor else None)
    in_names, out_names, out_avals, zero_outs = [], [], [], []
    for alloc in nc.m.functions[0].allocations:
        if not isinstance(alloc, mybir.MemoryLocationSet):
            continue
        name = alloc.memorylocations[0].name
        if alloc.kind == "ExternalInput":
            if name != partition_name:
                in_names.append(name)
        elif alloc.kind == "ExternalOutput":
            shape = tuple(alloc.tensor_shape)
            dtype = mybir.dt.np(alloc.dtype)
            out_names.append(name)
            out_avals.append(jax.core.ShapedArray(shape, dtype))
            zero_outs.append(np.zeros(shape, dtype))
    n_params = len(in_names)
    all_in = in_names + out_names
    if partition_name is not None:
        all_in = all_in + [partition_name]

    def _bodyfn(*args):
        operands = list(args)
        if partition_name is not None:
            operands.append(bass2jax.partition_id_tensor())
        outs = _bass_exec_p.bind(
            *operands, out_avals=tuple(out_avals), in_names=tuple(all_in),
            out_names=tuple(out_names), lowering_input_output_aliases=(),
            sim_require_finite=True, sim_require_nnan=True, nc=nc)
        return tuple(outs)

    try:
        devices = jax.devices("axon")[:N_CORES]
    except Exception:
        devices = jax.devices()[:N_CORES]
    mesh = Mesh(np.asarray(devices), ("core",))
    n_outs = len(out_names)
    fn = jax.jit(
        shard_map(_bodyfn, mesh=mesh,
                  in_specs=(PartitionSpec("core"),) * (n_params + n_outs),
                  out_specs=(PartitionSpec("core"),) * n_outs,
                  check_rep=False),
        keep_unused=True)
    sharding = jax.sharding.NamedSharding(mesh, PartitionSpec("core"))
    r = {"fn": fn, "in_names": in_names, "out_names": out_names,
         "zero_outs": zero_outs, "nc": nc, "sharding": sharding}
    _cache[key] = r
    return r


def _prep_host(x, W_enc, b_enc, W_dec, b_dec):
    x_eff = x - b_dec[None, :]
    xT_full = np.ascontiguousarray(x_eff.T, dtype=np.float32)      # [D, B]
    wencT = np.ascontiguousarray(W_enc.T, dtype=np.float32)        # [D, F]
    wdecT16 = np.ascontiguousarray(W_dec.T, dtype=np.float16)      # [F, D]
    beff = (b_enc.astype(np.float64)
            - W_enc.astype(np.float64) @ b_dec.astype(np.float64))
    beff = beff.astype(np.float32)[None, :]                        # [1, F]
    bdec = b_dec.astype(np.float32)[None, :]                       # [1, D]
    return xT_full, wencT, wdecT16, beff, bdec


def kernel(x, W_enc, b_enc, W_dec, b_dec, _repeat=1, _timeit=False):
    x = np.asarray(x, np.float32)
    W_enc = np.asarray(W_enc, np.float32)
    b_enc = np.asarray(b_enc, np.float32)
    W_dec = np.asarray(W_dec, np.float32)
    b_dec = np.asarray(b_dec, np.float32)
    xT_full, wencT, wdecT16, beff, bdec = _prep_host(x, W_enc, b_enc, W_dec, b_dec)
    has_benc = bool(np.any(beff))
    has_bdec = bool(np.any(b_dec))
    r = _get_runner(has_benc, has_bdec, _repeat)

    per_core = {
        "wdecT16": [wdecT16] * N_CORES,
        "beff": [beff] * N_CORES,
        "bdec": [bdec] * N_CORES,
    }
    per_core["xT"] = [np.ascontiguousarray(xT_full[:, c * BC:(c + 1) * BC])
                      for c in range(N_CORES)]
    per_core["wencT"] = [wencT] * N_CORES
    args = [np.concatenate(per_core[name], axis=0) for name in r["in_names"]]
    args += [np.concatenate([z] * N_CORES, axis=0) for z in r["zero_outs"]]

    import jax, time
    dev_args = [jax.device_put(a, r["sharding"]) for a in args]
    kernel.last_dev_args = dev_args
    kernel.last_runner = r
    outs = r["fn"](*dev_args)
    jax.block_until_ready(outs)
    if _timeit:
        times = []
        for _ in range(_timeit if isinstance(_timeit, int) and _timeit > 1 else 8):
            t0 = time.perf_counter()
            outs = r["fn"](*dev_args)
            jax.block_until_ready(outs)
            times.append(time.perf_counter() - t0)
        kernel.last_times = times

    xhat = np.asarray(outs[r["out_names"].index("xhat")])  # [B, D] concat
    return xhat.astype(np.float32)


# revision 27
# speedup vs baseline: 1.0696x; 1.0696x over previous
"""TopK sparse autoencoder forward pass on 8 Trainium2 NeuronCores.

Math (reference):
    preact = (x - b_dec) @ W_enc.T + b_enc          # [B, F]
    top32 = exact per-row top-32 of relu(preact)
    x_hat = scatter(top32) @ W_dec.T + b_dec        # [B, D]

Strategy: data-parallel over batch rows (1024 rows/core, no collectives).
Per core:
  encode: f32r matmul (PE, full rate at free-dim>=256); f-block-outer
          loop; PSUM -> stage (Act) -> DRAM fp32 spill + DVE chunk-max
          (width 32) into cm tiles.
  T1a (select): top-32 chunks via DVE max/max_index/match_replace rounds
          on cm [128,512]; issue 32 single-offset indirect DMAs (walrus
          in this container mislowers multi-offset forms and cannot load
          gpsimd libraries, so batched dma_gather is unavailable; each
          indirect DMA costs ~1us fixed on the Pool engine).
  T1b: 4 more DVE rounds on the 1024 gathered candidates give exact
          top-32 vals + global idx.
  T2a: build diag(vals) tiles (DVE) and pre-issue the first wdec-row
          gathers; T2b: remaining gathers + sum_c diag(vals[:,c]) @ G_c
          accumulated on the PE, then bias/copy + store.
Pipelining: engines are in-order, so the stages are software-pipelined
across b-tiles (gather of tile i+1 in flight while tile i runs its
DVE exact phase; PE stream stays pure encode until the group's encode
is done). Groups restream W_enc; earlier groups' tails hide inside the
next group's encode window.
"""
import sys
sys.path.insert(0, '/opt/trn_rl_repo')

import numpy as np

B, D, F, K = 8192, 768, 16384, 32
N_CORES = 8
BC = B // N_CORES          # rows per core (1024)
NBT = BC // 128            # b-tiles per core (8)
NFB = F // 512             # f-blocks (32)
KD = D // 128              # contraction chunks (6)
CH = 32                    # topk chunk width
C = F // CH                # chunks per row (512)
SH = CH.bit_length() - 1   # log2(CH)
NCH = K // 8               # rounds of 8 (4)

ENC_MODE = "f32r"          # "fp32" | "f32r"
TAIL_MODE = "dfirst"
GROUPS = (6, 2)            # b-tile group sizes (sum = NBT)
GP_BUFS = 10               # wdec gather buffers
PRE_G = 7                  # gathers pre-issued in t2a (2*PRE_G <= GP_BUFS)

_cache = {}


def _fix_sync_waits(nc, maxw=1):
    """This container's walrus rejects >1 sync wait per instruction; split
    excess waits onto same-engine NoOps inserted just before."""
    import bass_rust
    import concourse.mybir as mybir
    ctr = 0
    for f in nc.m.functions:
        for bb in f.blocks:
            out, changed = [], False
            for inst in bb.instructions:
                si = inst.sync_info
                waits = list(si.on_wait) if si is not None else []
                if len(waits) > maxw:
                    changed = True
                    head, keep = waits[:-maxw], waits[-maxw:]
                    for i in range(0, len(head), maxw):
                        ctr += 1
                        nop = mybir.InstNoOp(
                            name=f"syncfix-nop-{id(nc)}-{ctr}", ins=[], outs=[])
                        nop.engine = inst.engine
                        nop.sync_info = bass_rust.SyncInfo(
                            on_wait=head[i:i + maxw], on_update=[])
                        out.append(nop)
                    si.on_wait = keep
                out.append(inst)
            if changed:
                bb.instructions = out


def _build(has_benc: bool, has_bdec: bool, repeat: int = 1):
    import concourse.bass as bass
    import concourse.mybir as mybir
    import concourse.tile as tile
    dt = mybir.dt
    Alu = mybir.AluOpType

    nc = bass.Bass("TRN2", target_bir_lowering=False, debug=False,
                   num_devices=N_CORES)

    _edt = dt.float32r if ENC_MODE == "f32r" else dt.float32
    xT_d = nc.dram_tensor("xT", [D, BC], _edt, kind="ExternalInput")
    wencT_d = nc.dram_tensor("wencT", [D, F], _edt, kind="ExternalInput")
    wdecT_d = nc.dram_tensor("wdecT16", [F, D], dt.float16, kind="ExternalInput")
    beff_d = nc.dram_tensor("beff", [1, F], dt.float32, kind="ExternalInput")
    bdec_d = nc.dram_tensor("bdec", [1, D], dt.float32, kind="ExternalInput")
    out_d = nc.dram_tensor("xhat", [BC, D], dt.float32, kind="ExternalOutput")
    # one spill tensor per b-tile: gathers never falsely conflict (WAR)
    # with another tile's spill writes in the dep tracker
    preact_ds = [nc.dram_tensor(f"preact_spill{bt}", [128, F], dt.float32)
                 for bt in range(NBT)]
    preact_flats = [p.ap().rearrange("p (c w) -> (p c) w", w=CH)
                    for p in preact_ds]

    def body(tc, pools):
        sb, sb1, stp, psA, psB, gp, wbp = pools

        # resident inputs
        xT = sb1.tile([128, KD, BC],
                      dt.float32r if ENC_MODE == "f32r" else dt.float32)
        nc.sync.dma_start(
            xT[:], xT_d.ap().rearrange("(po pi) b -> pi po b", pi=128))
        if has_benc:
            beff = sb1.tile([1, F], dt.float32)
            nc.sync.dma_start(beff[:], beff_d.ap())
        if has_bdec:
            bdec1 = sb1.tile([1, D], dt.float32)
            nc.sync.dma_start(bdec1[:], bdec_d.ap())
            bdec_bc = sb1.tile([128, D], dt.float32)
            nc.gpsimd.partition_broadcast(bdec_bc[:], bdec1[:])

        cms = [sb1.tile([128, C], dt.float32, name=f"cm{bt}")
               for bt in range(NBT)]
        maxg = max(GROUPS)
        stgq = [[sb1.tile([128, 2, 512], dt.float32, name=f"stgq{loc}_{j}")
                 for j in range(2)] for loc in range(maxg)]
        loc_of = {}
        s0 = 0
        for n in GROUPS:
            for j in range(n):
                loc_of[s0 + j] = j
            s0 += n
        valss = [sb1.tile([128, K], dt.float32, name=f"vals{bt}")
                 for bt in range(NBT)]
        idxss = [sb1.tile([128, K], dt.uint32, name=f"idx{bt}")
                 for bt in range(NBT)]

        # iota constants (standard gpsimd library, preloaded)
        jiota = sb1.tile([128, K], dt.uint32)
        nc.gpsimd.iota(jiota[:], pattern=[[1, K]], base=0, channel_multiplier=0)
        # fp16 identity mask for building diag(vals) tiles on DVE
        iota_p = sb1.tile([128, 1], dt.uint32)
        nc.gpsimd.iota(iota_p[:], pattern=[[1, 1]], base=0, channel_multiplier=1)
        iota_f = sb1.tile([128, 128], dt.uint32)
        nc.gpsimd.iota(iota_f[:], pattern=[[1, 128]], base=0,
                       channel_multiplier=0)
        idmask = sb1.tile([128, 128], dt.float16)
        nc.vector.tensor_tensor(idmask[:],
                                iota_p[:, :1].to_broadcast([128, 128]),
                                iota_f[:], op=Alu.is_equal)
        # per-partition row offset into the per-tile gather table (p*C)
        rowoff = sb1.tile([128, 1], dt.uint32)
        nc.gpsimd.iota(rowoff[:], pattern=[[1, 1]], base=0,
                       channel_multiplier=C)

        # ---------------- encode + spill + chunk-max ----------------
        wencT_v = wencT_d.ap().rearrange("(po pi) f -> pi po f", pi=128)

        def encode_fb(fb, bts):
            wb = wbp.tile([128, KD, 512],
                          dt.float32r if ENC_MODE == "f32r" else dt.float32,
                          tag="wb")
            nc.sync.dma_start(wb[:], wencT_v[:, :, fb * 512:(fb + 1) * 512])
            if has_benc:
                beff_bc = stp.tile([128, 512], dt.float32, tag="beffbc")
                nc.gpsimd.partition_broadcast(
                    beff_bc[:], beff[:, fb * 512:(fb + 1) * 512])
            for bt in bts:
                ps = psA.tile([128, 512], dt.float32, tag="encps")
                for k in range(KD):
                    lhsT = xT[:, k, bt * 128:(bt + 1) * 128]
                    rhs = wb[:, k, :]
                    nc.tensor.matmul(ps[:], lhsT=lhsT, rhs=rhs,
                                     start=(k == 0), stop=(k == KD - 1))
                # evacuate PSUM via Act only (frees the bank fast; DVE
                # chunk-max reads the SBUF stage copy so encode never
                # stalls on DVE backlog). Two f-blocks accumulate in a
                # per-slot quad buffer; one 8KB-line spill DMA per quad.
                q = fb % 2
                sq = stgq[loc_of[bt]][(fb // 2) % 2]
                stage = sq[:, q]
                if has_benc:
                    nc.vector.tensor_add(stage, ps[:], beff_bc[:])
                else:
                    nc.scalar.copy(stage, ps[:])
                nc.vector.tensor_reduce(
                    cms[bt][:, fb * (512 // CH):(fb + 1) * (512 // CH)],
                    stage.rearrange("p (c w) -> p c w", w=CH),
                    axis=mybir.AxisListType.X, op=Alu.max)
                if q == 1:
                    nc.sync.dma_start(
                        preact_ds[bt].ap()[:, (fb - 1) * 512:(fb + 1) * 512],
                        sq[:].rearrange("p a b -> p (a b)"))

        # ---------------- T1: per-b-tile topk selection ----------------
        t1_state = {}

        def tail_select_a(bt):
            # top-32 chunks by chunk max; issue the candidate gathers
            cmw = sb.tile([128, C], dt.float32, tag="cmw")
            nc.vector.tensor_copy(cmw[:], cms[bt][:])
            cm8 = sb.tile([128, 8], dt.float32, tag="cm8")
            chunkid = sb.tile([128, K], dt.uint32, tag="chunkid",
                              name=f"chunkid{bt}")
            for r in range(NCH):
                nc.vector.max(out=cm8[:], in_=cmw[:])
                nc.vector.max_index(out=chunkid[:, r * 8:(r + 1) * 8],
                                    in_max=cm8[:], in_values=cmw[:])
                if r != NCH - 1:
                    nc.vector.match_replace(out=cmw[:], in_to_replace=cm8[:],
                                            in_values=cmw[:], imm_value=-1e30)

            off = sb.tile([128, K], dt.uint32, tag="off")
            nc.vector.tensor_tensor(off[:], chunkid[:],
                                    rowoff[:, :1].to_broadcast([128, K]),
                                    op=Alu.add)
            cand = sb.tile([128, K, CH], dt.float32, tag="cand",
                           name=f"cand{bt}")
            for j in range(K):
                nc.gpsimd.indirect_dma_start(
                    out=cand[:, j], out_offset=None,
                    in_=preact_flats[bt],
                    in_offset=bass.IndirectOffsetOnAxis(ap=off[:, j:j + 1],
                                                        axis=0))
            t1_state[bt] = (chunkid, cand)

        def tail_select_b(bt):
            # exact top-32 of the gathered candidates
            chunkid, cand = t1_state.pop(bt)
            candf = cand[:].rearrange("p a b -> p (a b)")
            vals = valss[bt]
            pos = sb.tile([128, K], dt.uint32, tag="pos")
            for r in range(NCH):
                m8 = vals[:, r * 8:(r + 1) * 8]
                nc.vector.max(out=m8, in_=candf)
                nc.vector.max_index(out=pos[:, r * 8:(r + 1) * 8],
                                    in_max=m8, in_values=candf)
                if r != NCH - 1:
                    nc.vector.match_replace(out=candf, in_to_replace=m8,
                                            in_values=candf, imm_value=-1e30)

            # positions -> global feature indices:
            # idx = chunkid[p, pos>>SH]*CH + (pos&(CH-1)); chunkid lookup via
            # one-hot compare-multiply-reduce (no per-partition gather on HW)
            j32 = sb.tile([128, K], dt.uint32, tag="j32")
            nc.vector.tensor_scalar(j32[:], pos[:], SH, None,
                                    op0=Alu.logical_shift_right)
            l32 = sb.tile([128, K], dt.uint32, tag="l32")
            nc.vector.tensor_scalar(l32[:], pos[:], CH - 1, None,
                                    op0=Alu.bitwise_and)
            eq = sb.tile([128, K, K], dt.uint32, tag="eq")
            nc.vector.tensor_tensor(
                eq[:], j32[:, :, None].to_broadcast([128, K, K]),
                jiota[:, None, :].to_broadcast([128, K, K]), op=Alu.is_equal)
            nc.vector.tensor_tensor(
                eq[:], eq[:], chunkid[:, None, :].to_broadcast([128, K, K]),
                op=Alu.mult)
            cs32 = sb.tile([128, K], dt.uint32, tag="cs32")
            nc.vector.tensor_reduce(cs32[:], eq[:],
                                    axis=mybir.AxisListType.X, op=Alu.max)
            idx32 = idxss[bt]
            nc.vector.tensor_scalar(idx32[:], cs32[:], SH, None,
                                    op0=Alu.logical_shift_left)
            nc.vector.tensor_tensor(idx32[:], idx32[:], l32[:], op=Alu.add)

            # relu guard (rows with <32 positive preacts: extra top-k entries
            # are relu zeros in the reference; zero coefficients match it)
            nc.vector.tensor_scalar_max(vals[:], vals[:], 0.0)

        # ---------------- T2: per-b-tile compact decode ----------------
        def tail_decode_b(bt):
            vals, idx32 = valss[bt], idxss[bt]
            # diag[p, c, :] = vals[p, c] * (identity row p)
            diag = sb.tile([128, K, 128], dt.float16, tag="diag",
                           name=f"diag{bt}")
            nc.vector.tensor_tensor(
                diag[:], vals[:, :, None].to_broadcast([128, K, 128]),
                idmask[:, None, :].to_broadcast([128, K, 128]), op=Alu.mult)
            pso = psB.tile([128, D], dt.float32, tag="decps")
            for c in range(K):
                g = gp.tile([128, D], dt.float16, tag="g")
                nc.gpsimd.indirect_dma_start(
                    out=g[:], out_offset=None,
                    in_=wdecT_d.ap(),
                    in_offset=bass.IndirectOffsetOnAxis(
                        ap=idx32[:, c:c + 1], axis=0))
                nc.tensor.matmul(pso[:, :512], lhsT=diag[:, c, :],
                                 rhs=g[:, :512],
                                 start=(c == 0), stop=(c == K - 1))
                nc.tensor.matmul(pso[:, 512:D], lhsT=diag[:, c, :],
                                 rhs=g[:, 512:D],
                                 start=(c == 0), stop=(c == K - 1))
            osb = stp.tile([128, D], dt.float32, tag="osb")
            if has_bdec:
                nc.vector.tensor_add(osb[:], pso[:], bdec_bc[:])
            else:
                nc.scalar.copy(osb[:], pso[:])
            nc.sync.dma_start(out_d.ap()[bt * 128:(bt + 1) * 128, :], osb[:])

        # ---------------- schedule ----------------
        groups, s = [], 0
        for n in GROUPS:
            groups.append(list(range(s, s + n)))
            s += n
        assert s == NBT

        stages = {"a": tail_select_a, "b": tail_select_b,
                  "d": tail_decode_b}
        prev, dq = [], []
        for bts in groups:
            # encode this group, interleaving the previous group's T1 stages
            # and T2 gather pre-issues between f-blocks, software-pipelined:
            # the candidate gathers of tile i+1 are in flight while tile i
            # runs its DVE exact phase. PE stream stays pure encode (decode
            # matmuls would stall the in-order PE queue on their gathers).
            ev = []
            for i, bt in enumerate(prev):
                ev.append(("a", bt))
                if i >= 1:
                    ev.append(("b", prev[i - 1]))
            if prev:
                ev.append(("b", prev[-1]))
            acts = {}
            if ev:
                step = max(1, (NFB - 1) // len(ev))
                for i, e in enumerate(ev):
                    acts.setdefault(1 + i * step, []).append(e)
            for fb in range(NFB):
                encode_fb(fb, bts)
                for kind, bt in acts.get(fb, ()):
                    stages[kind](bt)
            for kind, bt in [e for f, es in sorted(acts.items())
                             for e in es if f >= NFB]:
                stages[kind](bt)
            # decode of the group whose selection was just interleaved is
            # deferred into the final merged pipeline
            dq.extend(prev)
            prev = bts
        # final tail: interleave the deferred decodes with the last group's
        # selection pipeline so the Pool queue (gathers) never starves
        n = len(prev)
        ev = []
        if TAIL_MODE == "dfirst":
            # pending decodes first (Pool work immediately available), then
            # the last group's pipelined selection + its decodes
            ev.extend(("d", bt) for bt in dq)
            dq = []
            for i, bt in enumerate(prev):
                ev.append(("a", bt))
                if i >= 1:
                    ev.append(("b", prev[i - 1]))
            ev.append(("b", prev[-1]))
            ev.extend(("d", bt) for bt in prev)
        elif TAIL_MODE == "merge":
            for i, bt in enumerate(prev):
                ev.append(("a", bt))
                if i >= 1:
                    if dq:
                        ev.append(("d", dq.pop(0)))
                    ev.append(("b", prev[i - 1]))
                    dq.append(prev[i - 1])
                    if dq and i >= 2:
                        ev.append(("d", dq.pop(0)))
            ev.append(("b", prev[-1]))
            dq.append(prev[-1])
            ev.extend(("d", bt) for bt in dq)
        else:  # "simple": decodes of earlier groups, then last-group pipeline
            ev.extend(("d", bt) for bt in dq)
            dq = []
            for i in range(n + 2):
                if i < n:
                    ev.append(("a", prev[i]))
                if 1 <= i <= n:
                    ev.append(("b", prev[i - 1]))
                if i >= 2:
                    ev.append(("d", prev[i - 2]))
        for kind, bt in ev:
            stages[kind](bt)

    from contextlib import ExitStack
    with tile.TileContext(nc) as tc:
        with ExitStack() as ctx:
            pools = (
                ctx.enter_context(tc.tile_pool(name="sb", bufs=2)),
                ctx.enter_context(tc.tile_pool(name="sb1", bufs=1)),
                ctx.enter_context(tc.tile_pool(name="stage", bufs=4)),
                ctx.enter_context(tc.tile_pool(name="psA", bufs=6, space="PSUM")),
                ctx.enter_context(tc.tile_pool(name="psB", bufs=1, space="PSUM")),
                ctx.enter_context(tc.tile_pool(name="gpool", bufs=GP_BUFS)),
                ctx.enter_context(tc.tile_pool(name="wbpool", bufs=3)),
            )
            if repeat == 1:
                body(tc, pools)
            else:
                with tc.For_i(0, repeat, 1):
                    body(tc, pools)

    _fix_sync_waits(nc)
    return nc


def _get_runner(has_benc, has_bdec, repeat=1):
    key = (has_benc, has_bdec, repeat, ENC_MODE, GROUPS, CH, GP_BUFS, PRE_G)
    if key in _cache:
        return _cache[key]
    import jax
    from jax.sharding import Mesh, PartitionSpec
    from jax.experimental.shard_map import shard_map
    import concourse.mybir as mybir
    from concourse import bass2jax
    from concourse.bass2jax import _bass_exec_p, install_neuronx_cc_hook

    nc = _build(has_benc, has_bdec, repeat)
    install_neuronx_cc_hook()

    partition_name = (nc.partition_id_tensor.name
                      if nc.partition_id_tensor else None)
    in_names, out_names, out_avals, zero_outs = [], [], [], []
    for alloc in nc.m.functions[0].allocations:
        if not isinstance(alloc, mybir.MemoryLocationSet):
            continue
        name = alloc.memorylocations[0].name
        if alloc.kind == "ExternalInput":
            if name != partition_name:
                in_names.append(name)
        elif alloc.kind == "ExternalOutput":
            shape = tuple(alloc.tensor_shape)
            dtype = mybir.dt.np(alloc.dtype)
            out_names.append(name)
            out_avals.append(jax.core.ShapedArray(shape, dtype))
            zero_outs.append(np.zeros(shape, dtype))
    n_params = len(in_names)
    all_in = in_names + out_names
    if partition_name is not None:
        all_in = all_in + [partition_name]

    def _bodyfn(*args):
        operands = list(args)
        if partition_name is not None:
            operands.append(bass2jax.partition_id_tensor())
        outs = _bass_exec_p.bind(
            *operands, out_avals=tuple(out_avals), in_names=tuple(all_in),
            out_names=tuple(out_names), lowering_input_output_aliases=(),
            sim_require_finite=True, sim_require_nnan=True, nc=nc)
        return tuple(outs)

    try:
        devices = jax.devices("axon")[:N_CORES]
    except Exception:
        devices = jax.devices()[:N_CORES]
    mesh = Mesh(np.asarray(devices), ("core",))
    n_outs = len(out_names)
    fn = jax.jit(
        shard_map(_bodyfn, mesh=mesh,
                  in_specs=(PartitionSpec("core"),) * (n_params + n_outs),
                  out_specs=(PartitionSpec("core"),) * n_outs,
                  check_rep=False),
        keep_unused=True)
    sharding = jax.sharding.NamedSharding(mesh, PartitionSpec("core"))
    r = {"fn": fn, "in_names": in_names, "out_names": out_names,
         "zero_outs": zero_outs, "nc": nc, "sharding": sharding}
    _cache[key] = r
    return r


def _prep_host(x, W_enc, b_enc, W_dec, b_dec):
    x_eff = x - b_dec[None, :]
    xT_full = np.ascontiguousarray(x_eff.T, dtype=np.float32)      # [D, B]
    wencT = np.ascontiguousarray(W_enc.T, dtype=np.float32)        # [D, F]
    wdecT16 = np.ascontiguousarray(W_dec.T, dtype=np.float16)      # [F, D]
    beff = (b_enc.astype(np.float64)
            - W_enc.astype(np.float64) @ b_dec.astype(np.float64))
    beff = beff.astype(np.float32)[None, :]                        # [1, F]
    bdec = b_dec.astype(np.float32)[None, :]                       # [1, D]
    return xT_full, wencT, wdecT16, beff, bdec


def kernel(x, W_enc, b_enc, W_dec, b_dec, _repeat=1, _timeit=False):
    x = np.asarray(x, np.float32)
    W_enc = np.asarray(W_enc, np.float32)
    b_enc = np.asarray(b_enc, np.float32)
    W_dec = np.asarray(W_dec, np.float32)
    b_dec = np.asarray(b_dec, np.float32)
    xT_full, wencT, wdecT16, beff, bdec = _prep_host(x, W_enc, b_enc, W_dec, b_dec)
    has_benc = bool(np.any(beff))
    has_bdec = bool(np.any(b_dec))
    r = _get_runner(has_benc, has_bdec, _repeat)

    per_core = {
        "wdecT16": [wdecT16] * N_CORES,
        "beff": [beff] * N_CORES,
        "bdec": [bdec] * N_CORES,
    }
    per_core["xT"] = [np.ascontiguousarray(xT_full[:, c * BC:(c + 1) * BC])
                      for c in range(N_CORES)]
    per_core["wencT"] = [wencT] * N_CORES
    args = [np.concatenate(per_core[name], axis=0) for name in r["in_names"]]
    args += [np.concatenate([z] * N_CORES, axis=0) for z in r["zero_outs"]]

    import jax, time
    dev_args = [jax.device_put(a, r["sharding"]) for a in args]
    kernel.last_dev_args = dev_args
    kernel.last_runner = r
    outs = r["fn"](*dev_args)
    jax.block_until_ready(outs)
    if _timeit:
        times = []
        for _ in range(_timeit if isinstance(_timeit, int) and _timeit > 1 else 8):
            t0 = time.perf_counter()
            outs = r["fn"](*dev_args)
            jax.block_until_ready(outs)
            times.append(time.perf_counter() - t0)
        kernel.last_times = times

    xhat = np.asarray(outs[r["out_names"].index("xhat")])  # [B, D] concat
    return xhat.astype(np.float32)
